# revision 36
# baseline (speedup 1.0000x reference)
"""Trainium2 Bass kernel for MultiHeadLinearSelfAttention (linear attention +
LePE depthwise conv + projections), SPMD data-parallel over batch on 8 cores.

Design (~163us TimelineSim, vs 221us straightforward v1):
  - q/k/v projections run as 2-level fp8 DoubleRow matmuls (x and W each split
    into an fp8 term plus an fp8-encoded residual; scales chosen so all three
    term products land at a common 4x PSUM scale with no subnormal operands).
    K=256 contracts in one instruction at 0.5 cycles/row: ~2.6x fewer PE
    column-cycles than bf16 for these matmuls, ~0.1% added error.
  - The global 4x scale is absorbed for free: exp() takes scale=0.25 and a
    per-channel bias, clamps sit at 4, the emask entries are 0.25, and the
    attention normalization cancels the rest (q/k scale invariance).
  - q-path nonlinearity uses the exact identity
        4*(elu(z)+1) = min(4e^z, relu(4z+4b) + 4)   (since e^z >= 1+z):
    Act Exp(0.25*psum + (b+ln4)) in parallel with Act Relu(psum + 4b), then
    one DVE scalar_tensor_tensor min. The q bias rides the Act bias operands,
    eliminating the PE bias matmul of the q projection entirely (-6.8us PE).
    The k-path keeps its PE bias row matmul (pixel-major layout cannot take a
    per-channel Act bias) and the v1 exp/min/stt chain.
  - LePE 3x3: 7 taps as PE diag-matmuls accumulating into the attn PSUM,
    emitted BEFORE the qh-dependent block-diag kv matmul so PE fills the
    psum while DVE/Act still produce qh; 2 taps as a DVE MAC chain
    (scalar_tensor_tensor with per-channel weight pointers) over the padded
    v image, merged during PSUM evacuation.
  - v channel-major image built by XBAR dma_start_transpose (idle DMA/SP
    engines) + one DVE tensor_scalar (0.25x rescale + v-bias) into the
    zero-ringed 66x66 padded image; no second v projection pass.
  - ksum rides the kv PSUM tile as column 128 of the same accumulation group
    (one accumulation group per 2KB PSUM bank is a hardware rule).
  - Software-pipelined emission everywhere (engines execute their streams
    in order): phase A lags the kv matmuls 8 iterations behind kvpix and
    rotates kvpix through 6 PSUM banks; phase C is a 5-stage pipeline
    (q/elu -> den/recip -> bc/qh -> attn+LePE -> out), so no engine
    head-of-line blocks on a cross-engine round trip.
  - v-bias is not applied in the attention path: sum_d qhat*ksum == 1 per
    head makes it a constant per-channel shift folded into the output bias
    on the host. The LePE path sees the true biased v.
  - HW constraint honored throughout: GPSIMD/Pool supports only SBUF
    tensor_scalar ops (no tensor_tensor / scalar_tensor_tensor, no PSUM).
"""

import os
import sys

for _p in ("/opt/trn_rl_repo",):
    if _p not in sys.path:
        sys.path.insert(0, _p)

import numpy as np
import ml_dtypes

import concourse.bass as bass
import concourse.bacc as bacc
import concourse.mybir as mybir
import concourse.tile as tile
from concourse.bass_utils import run_bass_kernel_spmd

BF16 = mybir.dt.bfloat16
F32 = mybir.dt.float32
AF = mybir.ActivationFunctionType
ALU = mybir.AluOpType

N_CORES = 8
IMG = 2            # images per core (B=16)
C = 256
N = 4096           # pixels (64x64)
G = 2              # channel groups of 128
TAPS = [(ty - 1, tx - 1) for ty in range(3) for tx in range(3)]

# NOTE: GPSIMD (pool) cannot access PSUM on hardware — pool only gets
# SBUF->SBUF ops (min-combines, LePE sub-chain); PSUM readers go to dve/act.
NPE = int(os.environ.get("BK_NPE", "7"))       # LePE taps on PE (rest off-PE)
EV_VSB = os.environ.get("BK_VSB", "mix")       # v psum->sbuf evac: act|dve|mix
EV_RK = os.environ.get("BK_RK", "dve")         # k relu+1 (reads psum)
EV_RQ = os.environ.get("BK_RQ", "dve")         # q relu+1 (reads psum)
EV_MINK = os.environ.get("BK_MINK", "pool")    # k min-combine (sbuf)
EV_MINQ = os.environ.get("BK_MINQ", "pool")    # q min-combine (sbuf)
EV_QH = os.environ.get("BK_QH", "dve")         # qh = S*bc (reads psum)
EV_OT1 = os.environ.get("BK_OT1", "act")       # out evac for o=1 (reads psum)
EV_CHB = os.environ.get("BK_CHB", "dve")      # LePE sub-chain B engine (sbuf)

_CACHE = {}


def build_program():
    nc = bacc.Bacc(
        "TRN2", target_bir_lowering=False, debug=False,
        enable_asserts=False, num_devices=N_CORES,
    )
    F8 = mybir.dt.float8e4
    DR = mybir.MatmulPerfMode.DoubleRow
    x_d = nc.dram_tensor("x", [IMG, 3, 128, 2, N], F8, kind="ExternalInput").ap()
    wkvdr_d = nc.dram_tensor("wkvdr", [3, 128, 2, 512], F8,
                             kind="ExternalInput").ap()
    wqdr_d = nc.dram_tensor("wqdr", [3, 128, 2, 256], F8,
                            kind="ExternalInput").ap()
    wpack_d = nc.dram_tensor("wpack", [G, 128, 2192], BF16, kind="ExternalInput").ap()
    brow_d = nc.dram_tensor("brow", [1, 512], BF16, kind="ExternalInput").ap()
    bcol_d = nc.dram_tensor("bcol", [C, 15], F32, kind="ExternalInput").ap()
    em_d = nc.dram_tensor("emask", [8, 256], BF16, kind="ExternalInput").ap()
    y_d = nc.dram_tensor("y", [IMG, C, N], F32, kind="ExternalOutput").ap()

    pe_taps = list(range(NPE))
    dve_taps = list(range(NPE, 9))

    def eng(name):
        return {"pool": nc.gpsimd, "dve": nc.vector}[name]

    def relu1_op(engine, dst, src, bias1):
        # dst = max(src + bias1, 1) == relu(src + bias1 - 1) + 1
        eng(engine).tensor_scalar(dst, src, bias1, 1.0, ALU.add, ALU.max)

    def min_op(engine, dst, a, b):
        eng(engine).tensor_tensor(dst, a, b, ALU.min)

    def mul_op(engine, dst, a, b):
        eng(engine).tensor_tensor(dst, a, b, ALU.mult)

    def copy_op(engine, dst, src):
        if engine == "act":
            nc.scalar.copy(dst, src)
        else:
            eng(engine).tensor_copy(dst, src)

    with tile.TileContext(nc) as tc:
        with (
            tc.tile_pool(name="const", bufs=1) as const,
            tc.tile_pool(name="sb", bufs=1) as sb,
            tc.tile_pool(name="ps", bufs=1, space=bass.MemorySpace.PSUM) as ps,
        ):
            # ---------------- constants ----------------
            wq, wkv, wo, lep, wcol = [], [], [], [], []
            wp = []
            for g in range(G):
                t = const.tile([128, 2192], BF16, name=f"wp{g}", tag=f"wp{g}")
                nc.scalar.dma_start(t[:], wpack_d[g])
                wp.append(t)
                wq.append(t[:, 0:256])
                wkv.append(t[:, 256:768])
                wo.append(t[:, 768:1024])
                lep.append([t[:, 1024 + ti * 128:1024 + (ti + 1) * 128]
                            for ti in range(9)])
                wcol.append([t[:, 2176 + ti:2177 + ti] for ti in range(9)])
            wkvdr, wqdr = [], []
            for t_ in range(3):
                a = const.tile([128, 2, 512], F8, name=f"wkvdr{t_}",
                               tag=f"wkvdr{t_}")
                nc.scalar.dma_start(a[:], wkvdr_d[t_])
                wkvdr.append(a)
                b = const.tile([128, 2, 256], F8, name=f"wqdr{t_}",
                               tag=f"wqdr{t_}")
                nc.scalar.dma_start(b[:], wqdr_d[t_])
                wqdr.append(b)
            emt = const.tile([8, 256], BF16, name="emt", tag="emt")
            nc.sync.dma_start(emt[:], em_d[:])
            em = [emt[:, 0:128], emt[:, 128:256]]
            brow = const.tile([1, 512], BF16, name="brow", tag="brow")
            nc.sync.dma_start(brow[:], brow_d[:])
            bkr = brow[:, 0:256]
            bqr = [brow[:, 256 + o * 128:256 + (o + 1) * 128] for o in range(2)]
            bct = []
            for g in range(G):
                t = const.tile([128, 15], F32, name=f"bct{g}", tag=f"bct{g}")
                nc.sync.dma_start(t[:], bcol_d[g * 128:(g + 1) * 128, :])
                bct.append(t)
            bq = [bct[g][:, 0:1] for g in range(G)]
            bq1 = [bct[g][:, 1:2] for g in range(G)]
            bvc = [bct[g][:, 2:3] for g in range(G)]
            btc = [bct[g][:, 3:4] for g in range(G)]
            wcolf = [[bct[g][:, 4 + ti:5 + ti] for ti in range(9)]
                     for g in range(G)]
            bq4 = [bct[g][:, 13:14] for g in range(G)]
            bq44 = [bct[g][:, 14:15] for g in range(G)]
            EV_RELU = os.environ.get("BK_RELU", "act")
            ones512 = const.tile([1, 512], BF16, name="ones512", tag="ones512")
            nc.gpsimd.memset(ones512[:], 1.0)
            onesr = const.tile([1, 128], BF16, name="onesr", tag="onesr")
            nc.gpsimd.memset(onesr[:], 1.0)
            onesc = const.tile([128, 1], BF16, name="onesc", tag="onesc")
            nc.gpsimd.memset(onesc[:], 1.0)
            ln4c = const.tile([128, 1], F32, name="ln4c", tag="ln4c")
            nc.gpsimd.memset(ln4c[:], float(np.log(4.0)))

            st = {}   # per-image state

            # ---------------- phases ----------------
            def load_x(u, img0):
                # chunk-major interleave: the first kvpix only needs the
                # leading pixel chunk of all three fp8 terms, so compute
                # starts ~4us earlier than with three whole-image DMAs.
                xs = [sb.tile([128, 2, N], F8, name=f"x{u}_{t_}",
                              tag=f"x{t_}", bufs=2) for t_ in range(3)]
                NCH = int(os.environ.get("BK_NCH", "2"))
                csz = N // NCH
                for c_ in range(NCH):
                    csl = slice(c_ * csz, (c_ + 1) * csz)
                    for t_ in range(3):
                        nc.sync.dma_start(xs[t_][:, :, csl],
                                          x_d[img0, t_][:, :, csl])
                st[u] = {"xs": xs}

            def phase_a(u):
                xs = st[u]["xs"]
                # col 128 accumulates ksum in the same PSUM accumulation
                # group as kv (one group per 2KB bank is a hardware rule)
                kvp = [ps.tile([128, 129], F32, name=f"kv{u}_{g}",
                               tag=f"kv{g}", bufs=1) for g in range(G)]
                st[u]["kvp"] = kvp
                vpad = []
                for g in range(G):
                    t = sb.tile([128, 66 * 66], BF16, name=f"vp{u}_{g}",
                                tag=f"vp{g}", bufs=2)
                    t3 = t[:].rearrange("p (r x) -> p r x", x=66)
                    nc.gpsimd.memzero(t3[:, 0, :])
                    nc.gpsimd.memzero(t3[:, 65, :])
                    nc.gpsimd.memzero(t3[:, 1:65, 0:2])
                    nc.gpsimd.memzero(t3[:, 1:65, 64:66])
                    vpad.append(t)
                st[u]["vpad"] = vpad
                # software-pipelined emission: the kv/ksum matmuls of
                # iteration j-1 are emitted after the kvpix matmuls of j, so
                # the PE stream never head-of-line blocks on the elementwise
                # round-trip producing kh.
                khs, vchunks = {}, {}
                vchunk = None
                LAG_A = int(os.environ.get("BK_LAGA", "8"))
                for j in range(32 + LAG_A):
                    if j < 32:
                        jc, jj = j // 8, j % 8
                        if jj == 0:
                            vchunk = sb.tile([128, 8, 256], BF16,
                                             name=f"vc{u}_{jc}", tag="vch",
                                             bufs=3)
                            vchunks[jc] = vchunk
                        kvpix = ps.tile([128, 512], F32, name=f"kvpix{u}_{j}",
                                        tag=("big" if j % 2 == 0 else "attn"),
                                        bufs=3)
                        for t_ in range(3):
                            nc.tensor.matmul(
                                kvpix[:], xs[t_][:, :, j * 128:(j + 1) * 128],
                                wkvdr[t_][:], start=(t_ == 0), stop=False,
                                perf_mode=DR)
                        nc.tensor.matmul(kvpix[:, 0:256], onesr[:], bkr,
                                         start=False, stop=True)
                        ek = sb.tile([128, 256], BF16, name=f"ek{u}_{j}",
                                     tag="ek", bufs=8)
                        nc.scalar.activation(ek[:], kvpix[:, 0:256], AF.Exp,
                                             scale=0.25, bias=ln4c[:])
                        gk = sb.tile([128, 256], BF16, name=f"gk{u}_{j}",
                                     tag="rk", bufs=8)
                        nc.gpsimd.tensor_scalar_min(gk[:], ek[:], 4.0)
                        kh = sb.tile([128, 256], BF16, name=f"kh{u}_{j}",
                                     tag="kh", bufs=8)
                        nc.vector.scalar_tensor_tensor(kh[:], kvpix[:, 0:256],
                                                       0.0, gk[:], ALU.max,
                                                       ALU.add)
                        khs[j] = kh
                        if EV_VSB == "mix":
                            ve = "act" if j % 2 else "dve"
                        elif EV_VSB == "mix2":
                            ve = "act" if j % 5 < 3 else "dve"
                        else:
                            ve = EV_VSB
                        copy_op(ve, vchunk[:, jj, :], kvpix[:, 256:512])
                    if j >= LAG_A:
                        pj = j - LAG_A
                        pjc, pjj = pj // 8, pj % 8
                        pkh, pvc = khs.pop(pj), vchunks[pjc]
                        for g in range(G):
                            gs = slice(g * 128, (g + 1) * 128)
                            nc.tensor.matmul(kvp[g][:, 0:128], pkh[:, gs],
                                             pvc[:, pjj, gs],
                                             start=(pj == 0), stop=False)
                            nc.tensor.matmul(kvp[g][:, 128:129], pkh[:, gs],
                                             onesc[:],
                                             start=False, stop=(pj == 31))
                        if pjj == 7:
                            vt = sb.tile([128, 16, 128], BF16,
                                         name=f"vt{u}_{pjc}", tag="vt", bufs=3)
                            nc.sync.dma_start_transpose(vt[:], vchunks[pjc][:])
                            vt5 = vt[:].rearrange(
                                "p (j two) (r x) -> p j two r x", two=2, x=64)
                            for g in range(G):
                                vp3 = vpad[g][:].rearrange("p (r x) -> p r x",
                                                           x=66)
                                dst = vp3[:, 1 + 16 * pjc:1 + 16 * (pjc + 1),
                                          1:65]
                                dst4 = dst.rearrange("p (j r) x -> p j r x", r=2)
                                nc.vector.tensor_scalar(dst4, vt5[:, :, g, :, :],
                                                        0.25, bvc[g],
                                                        ALU.mult, ALU.add)

            def phase_b(u):
                kvp = st[u]["kvp"]
                bd, KS = [], []
                for g in range(G):
                    b = sb.tile([128, 128], BF16, name=f"bd{u}_{g}", tag=f"bd{g}",
                                bufs=2)
                    nc.gpsimd.memzero(b[:])
                    ks = sb.tile([128, 8], BF16, name=f"KS{u}_{g}", tag=f"KS{g}",
                                 bufs=2)
                    nc.gpsimd.memzero(ks[:])
                    for h in range(4):
                        sl = slice(h * 32, (h + 1) * 32)
                        if h % 2 == 0:
                            nc.scalar.copy(b[sl, sl], kvp[g][sl, sl])
                            nc.vector.tensor_copy(
                                ks[sl, g * 4 + h:g * 4 + h + 1],
                                kvp[g][sl, 128:129])
                        else:
                            nc.vector.tensor_copy(b[sl, sl], kvp[g][sl, sl])
                            nc.scalar.copy(ks[sl, g * 4 + h:g * 4 + h + 1],
                                           kvp[g][sl, 128:129])
                    bd.append(b)
                    KS.append(ks)
                st[u]["bd"], st[u]["KS"] = bd, KS

            cst = {}   # per-(u, nt) c-phase state

            def c_stage_q(u, nt):
                xs = st[u]["xs"]
                nsl = slice(nt * 512, (nt + 1) * 512)
                Sq = []
                for o in range(G):
                    osl = slice(o * 128, (o + 1) * 128)
                    qp = ps.tile([128, 512], F32, name=f"qp{u}_{o}_{nt}",
                                 tag="big", bufs=3)
                    for t_ in range(3):
                        nc.tensor.matmul(qp[:], wqdr[t_][:, :, osl],
                                         xs[t_][:, :, nsl],
                                         start=(t_ == 0), stop=(t_ == 2),
                                         perf_mode=DR)
                    eq = sb.tile([128, 512], BF16, name=f"eq{u}_{o}_{nt}",
                                 tag="eq", bufs=6)
                    nc.scalar.activation(eq[:], qp[:], AF.Exp, scale=0.25,
                                         bias=bq[o])
                    gq = sb.tile([128, 512], BF16, name=f"gq{u}_{o}_{nt}",
                                 tag="rq", bufs=6)
                    S = sb.tile([128, 512], BF16, name=f"S{u}_{o}_{nt}",
                                tag=f"S{o}", bufs=7)
                    # S4 = 4*(elu(z)+1) = min(4e^z, relu(4z+4b)+4), exact
                    # since e^z >= 1+z; the bias rides the scalar operands so
                    # no PE bias matmul is needed. (HW Pool does not support
                    # tensor_tensor/stt, so both steps live on DVE/Act.)
                    ev = EV_RELU if EV_RELU != "mix" else ("dve" if o == 0
                                                           else "act")
                    if ev == "act":
                        nc.scalar.activation(gq[:], qp[:], AF.Relu, bias=bq4[o])
                        nc.vector.scalar_tensor_tensor(S[:], gq[:], 4.0,
                                                       eq[:], ALU.add, ALU.min)
                    else:
                        nc.vector.tensor_scalar(gq[:], qp[:], bq44[o], 4.0,
                                                ALU.add, ALU.max)
                        nc.vector.tensor_tensor(S[:], eq[:], gq[:], ALU.min)
                    Sq.append(S)
                cst[(u, nt)] = {"Sq": Sq}

            def c_stage_den(u, nt):
                KS = st[u]["KS"]
                Sq = cst[(u, nt)]["Sq"]
                den = ps.tile([128, 512], F32, name=f"den{u}_{nt}", tag="kv0",
                              bufs=1)
                nc.tensor.matmul(den[0:8, :], KS[0][:], Sq[0][:], start=True,
                                 stop=False)
                nc.tensor.matmul(den[0:8, :], KS[1][:], Sq[1][:], start=False,
                                 stop=True)
                rc = sb.tile([8, 512], BF16, name=f"rc{u}_{nt}", tag="rc", bufs=6)
                with nc.allow_low_precision(reason="recip feeds bf16 matmul"):
                    nc.vector.reciprocal(rc[:], den[0:8, :])
                cst[(u, nt)]["rc"] = rc

            def c_stage_bc(u, nt):
                Sq, rc = cst[(u, nt)]["Sq"], cst[(u, nt)]["rc"]
                qhs = []
                for g in range(G):
                    bc = ps.tile([128, 512], F32, name=f"bc{u}_{g}_{nt}",
                                 tag=("kv1" if g == 0 else "kv0"), bufs=1)
                    nc.tensor.matmul(bc[:], em[g], rc[:], start=True, stop=True)
                    qh = sb.tile([128, 512], BF16, name=f"qh{u}_{g}_{nt}",
                                 tag=f"qh{g}", bufs=5)
                    mul_op(EV_QH, qh[:], Sq[g][:], bc[:])
                    qhs.append(qh)
                cst[(u, nt)]["qh"] = qhs

            def c_stage_at(u, nt):
                vpad, bd = st[u]["vpad"], st[u]["bd"]
                qhs = cst[(u, nt)]["qh"]
                rats = []
                for g in range(G):
                    at = ps.tile([128, 512], F32, name=f"at{u}_{g}_{nt}",
                                 tag="attn", bufs=3)
                    vp3 = vpad[g][:].rearrange("p (r x) -> p r x", x=66)
                    for i, ti in enumerate(pe_taps):
                        dy, dx = TAPS[ti]
                        inap = vp3[:, 8 * nt + dy + 1:8 * nt + dy + 9,
                                   1 + dx:1 + dx + 64]
                        nc.tensor.matmul(at[:], lep[g][ti], inap,
                                         start=(i == 0), stop=False)
                    nc.tensor.matmul(at[:], bd[g][:], qhs[g][:],
                                     start=(not pe_taps), stop=True)
                    rat = sb.tile([128, 512], BF16, name=f"rat{u}_{g}_{nt}",
                                  tag=f"rat{g}", bufs=6)
                    if os.environ.get("BK_ATEVAC", "1") == "1" and dve_taps:
                        atb = sb.tile([128, 512], BF16,
                                      name=f"atb{u}_{g}_{nt}", tag="atb",
                                      bufs=4)
                        nc.scalar.copy(atb[:], at[:])
                        at = atb

                    def tap_ap(ti):
                        dy, dx = TAPS[ti]
                        return vp3[:, 8 * nt + dy + 1:8 * nt + dy + 9,
                                   1 + dx:1 + dx + 64]

                    def chtile(i):
                        return sb.tile([128, 512], BF16,
                                       name=f"ch{u}_{g}_{nt}_{i}",
                                       tag="ch", bufs=6)

                    if not dve_taps:
                        nc.scalar.copy(rat[:], at[:])
                    else:
                        # two parallel sub-chains to shorten the serial
                        # latency: A starts from the attn psum, B from a
                        # cheap TSP; merged by a final tensor_tensor add.
                        half = (len(dve_taps) + 1) // 2
                        ca, cb = dve_taps[:half], dve_taps[half:]
                        prev = at
                        for i, ti in enumerate(ca):
                            dst = rat if not cb and i == len(ca) - 1 else chtile(i)
                            nc.vector.scalar_tensor_tensor(
                                dst[:], tap_ap(ti), wcolf[g][ti], prev[:],
                                ALU.mult, ALU.add)
                            prev = dst
                        if cb:
                            be = eng(EV_CHB)
                            prevb = chtile(8)
                            be.tensor_scalar(prevb[:], tap_ap(cb[0]),
                                             wcolf[g][cb[0]], None, ALU.mult)
                            for i, ti in enumerate(cb[1:]):
                                dst = chtile(4 + i)
                                be.scalar_tensor_tensor(
                                    dst[:], tap_ap(ti), wcolf[g][ti], prevb[:],
                                    ALU.mult, ALU.add)
                                prevb = dst
                            nc.vector.tensor_tensor(rat[:], prev[:], prevb[:],
                                                    ALU.add)
                    rats.append(rat)
                cst[(u, nt)]["rat"] = rats

            def c_stage_out(u, nt):
                img0 = u
                nsl = slice(nt * 512, (nt + 1) * 512)
                rats = cst[(u, nt)]["rat"]
                for o in range(G):
                    osl = slice(o * 128, (o + 1) * 128)
                    op_ = ps.tile([128, 512], F32, name=f"op{u}_{o}_{nt}",
                                  tag="attn", bufs=3)
                    nc.tensor.matmul(op_[:], wo[0][:, osl], rats[0][:],
                                     start=True, stop=False)
                    nc.tensor.matmul(op_[:], wo[1][:, osl], rats[1][:],
                                     start=False, stop=True)
                    ot = sb.tile([128, 512], F32, name=f"ot{u}_{o}_{nt}",
                                 tag=f"ot{o}", bufs=2)
                    if o == 0 or EV_OT1 == "act":
                        nc.scalar.activation(ot[:], op_[:], AF.Identity,
                                             bias=btc[o])
                    else:
                        eng(EV_OT1).tensor_scalar_add(ot[:], op_[:], btc[o])
                    nc.sync.dma_start(y_d[img0, osl, nsl], ot[:])
                del cst[(u, nt)]

            # ---------------- schedule ----------------
            # software-pipelined c-phase: stage s of iteration i is emitted at
            # round i+s, so every PE instruction's inputs were produced >=1
            # round earlier and no engine head-of-line blocks.
            for i in range(IMG):
                load_x(i, i)
            for u in range(IMG):
                phase_a(u)
                phase_b(u)
            IT = [(u, nt) for nt in range(8) for u in range(IMG)]
            STAGES = [c_stage_q, c_stage_den, c_stage_bc, c_stage_at,
                      c_stage_out]
            OFF = [int(v) for v in os.environ.get("BK_OFF", "0,1,2,3,4").split(",")]
            for r in range(len(IT) + OFF[-1]):
                for s, fn in enumerate(STAGES):
                    i = r - OFF[s]
                    if 0 <= i < len(IT):
                        fn(*IT[i])

    nc.compile()
    return nc


def _dr(arr):
    """[256, M] -> [128, 2, M] DoubleRow layout."""
    return np.stack([arr[0:128], arr[128:256]], axis=1)


def _split3(arr):
    """2-level fp8 split of [256, M] weights at global psum scale 4."""
    f8 = ml_dtypes.float8_e4m3
    w4 = 4.0 * arr
    w1 = w4.astype(f8)
    w1f = w1.astype(np.float32)
    w2 = (w1f / 4.0).astype(f8)
    w3 = (16.0 * (w4 - w1f)).astype(f8)
    return [_dr(w1), _dr(w2), _dr(w3)]


def _prep_inputs(x, qkv_w, qkv_b, lepe_w, lepe_b, out_w, out_b):
    bf = ml_dtypes.bfloat16
    f8 = ml_dtypes.float8_e4m3
    x = np.ascontiguousarray(np.asarray(x, np.float32)).reshape(16, C, N)
    qkv_w = np.asarray(qkv_w, np.float32)
    qkv_b = np.asarray(qkv_b, np.float32)
    lepe_w = np.asarray(lepe_w, np.float32)
    lepe_b = np.asarray(lepe_b, np.float32)
    out_w = np.asarray(out_w, np.float32)
    out_b = np.asarray(out_b, np.float32)

    wqT = qkv_w[0:256].T
    wkvT = np.concatenate([qkv_w[256:512].T, qkv_w[512:768].T], axis=1)
    woT = out_w.T
    lepe = np.zeros((G, 9, 128, 128), np.float32)
    for g in range(G):
        for ti, (dy, dx) in enumerate(TAPS):
            wcol = lepe_w[g * 128:(g + 1) * 128, 0, dy + 1, dx + 1]
            np.fill_diagonal(lepe[g, ti], wcol)
    wpack = np.zeros((G, 128, 2192), np.float32)
    for g in range(G):
        sl = slice(g * 128, (g + 1) * 128)
        wpack[g, :, 0:256] = wqT[sl]
        wpack[g, :, 256:768] = wkvT[sl]
        wpack[g, :, 768:1024] = woT[sl]
        wpack[g, :, 1024:2176] = lepe[g].transpose(1, 0, 2).reshape(128, 1152)
        for ti, (dy, dx) in enumerate(TAPS):
            wpack[g, :, 2176 + ti] = lepe_w[sl.start:sl.stop, 0, dy + 1, dx + 1]
    wpack = wpack.astype(bf)
    bv = qkv_b[512:768]
    # v bias is not applied in the attention path: since sum_d qhat*ksum == 1
    # per head, its effect is a constant per-channel shift folded into the
    # output bias here. The LePE path sees the true v (bias added on the
    # vpad write).
    btotc = out_b + out_w @ lepe_b + out_w @ bv
    # psum carries 4x-scaled q/k/v; bias rows/cols follow (see build_program)
    brow = np.concatenate([4.0 * qkv_b[256:512],
                          4.0 * qkv_b[0:256]]).reshape(1, 512).astype(bf)
    bq = qkv_b[0:256]
    wtaps = [lepe_w[:, 0, dy + 1, dx + 1] for (dy, dx) in TAPS]
    bcol = np.stack([bq + np.log(4.0), 4.0 * (bq + 1.0), bv, btotc]
                    + wtaps + [4.0 * bq, 4.0 * bq + 4.0],
                    axis=1).astype(np.float32)
    emask = np.zeros((8, 256), np.float32)
    for e in range(256):
        emask[e // 32, (e // 128) * 128 + e % 128] = 0.25
    emask = emask.astype(bf)

    wkvdr = np.stack(_split3(wkvT))
    wqdr = np.stack(_split3(wqT))

    # x 2-level fp8 terms, DR layout [IMG, 3, 128, 2, N]
    x1 = x.astype(f8)
    x2 = (4.0 * (x - x1.astype(np.float32))).astype(f8)
    x3 = (x / 16.0).astype(f8)
    xdr = np.zeros((16, 3, 128, 2, N), f8)
    for t_, xt in enumerate((x1, x2, x3)):
        xdr[:, t_] = np.stack([xt[:, 0:128], xt[:, 128:256]], axis=2)

    shared = dict(wpack=wpack, brow=brow, bcol=bcol, emask=emask,
                  wkvdr=wkvdr, wqdr=wqdr)
    in_maps = []
    for c in range(N_CORES):
        m = dict(shared)
        m["x"] = xdr[c * IMG:(c + 1) * IMG]
        in_maps.append(m)
    return in_maps


def kernel(x, qkv_w, qkv_b, lepe_w, lepe_b, out_w, out_b):
    if "nc" not in _CACHE:
        _CACHE["nc"] = build_program()
    nc = _CACHE["nc"]
    in_maps = _prep_inputs(x, qkv_w, qkv_b, lepe_w, lepe_b, out_w, out_b)
    res = run_bass_kernel_spmd(nc, in_maps, core_ids=list(range(N_CORES)))
    out = np.concatenate([np.asarray(r["y"], np.float32) for r in res.results])
    return out.reshape(16, C, 64, 64)


if __name__ == "__main__":
    build_program()
    print("build OK")

